# revision 34
# baseline (speedup 1.0000x reference)
"""Multi-head attention (B=2, N=2048, C=1024, H=16, qk-RMSNorm) on 8 TRN2 cores.

v3 of the kernel. Same sharding as baseline (TP over 4 head groups x DP
over batch; host sums the 4 w_proj partials per batch) and the same
attention pipeline (S(cur) || AV(prev), then norm(prev2) + proj).

Changes vs v2 (299.98us):
- fp16 everywhere 16-bit data goes: x, w_qkv, w_proj, qkT, pt, v_aug,
  attnT, sums. Same PE rate as fp32r/bf16, half the DMA + LDWEIGHTS
  bytes of the fp32r GEMM weights, and 4x less quantization noise than
  bf16 (fp16 has 10 mantissa bits vs bf16's 8; all magnitudes here are
  well inside fp16 range: |logits*scale| < ~6 -> pt < ~450 < 65504).
- v_aug is produced directly by the v GEMM: w_v is host-padded to
  [C, HL*65] with zero columns at the per-head ones positions and the
  bias row carries 1.0 there, so acc = x@wv_aug + ones*bv_aug lands in
  the augmented layout. One DVE copy per key block replaces the 4
  scalar.copy + 4 gpsimd ones-writes of v2 (~30us of ACT/Pool work).
- RMSNorm: the two heads of an m-slice share one ssq PSUM tile (the
  second head's sumsq matmul is tile-positioned at partition 64), so
  sqrt / reciprocal / scale-mul run once per m-slice on [128, nq]
  instead of per head on [64, nq]: halves those ACT/DVE instr counts.
  sq is computed from the fp16 qkT in SBUF (2x DVE mode) instead of
  from PSUM.

PSUM budget unchanged: tag "s2" [128,1024] bufs=2, tag "oas" [128,512]
bufs=4.
"""

import sys

if "/opt/trn_rl_repo" not in sys.path:
    sys.path.insert(0, "/opt/trn_rl_repo")

from contextlib import ExitStack

import numpy as np

import concourse.mybir as mybir
import concourse.tile as tile
from concourse import bacc
from concourse.bass_utils import run_bass_kernel_spmd

F32 = mybir.dt.float32
F16 = mybir.dt.float16
AF = mybir.ActivationFunctionType

B, N, C, H = 2, 2048, 1024, 16
D = C // H          # 64
EPS = 1e-6
NCORES = 8
GROUPS = 4          # head groups (cores per batch)
HL = H // GROUPS    # heads per core = 4
CL = HL * D         # local channels = 256
SCALE = D ** -0.5   # 0.125

P = 128             # partition dim
KT = C // P         # 8 contraction tiles over C
NQ = 512            # query/token block
HPB = P // D        # heads per 128-channel block = 2
VW = D + 1          # 65: v columns + ones column
CLA = HL * VW       # 260: augmented v width


def build(n=N, nq=NQ):
    nb = n // P          # key blocks of 128
    nj = n // nq         # token blocks of nq
    kt = KT

    nc = bacc.Bacc("TRN2", target_bir_lowering=False, debug=False,
                   num_devices=NCORES)

    # inputs are host-pretiled so each loads with a single wide DMA
    # (DMA triggers serialize at ~600ns apiece on the sync queue)
    xT_d = nc.dram_tensor("xT", [P, KT, n], F16, kind="ExternalInput").ap()
    wqk_d = nc.dram_tensor("w_qk", [P, KT, 2 * CL], F16, kind="ExternalInput").ap()
    wv_d = nc.dram_tensor("w_v", [P, KT, CL], F16, kind="ExternalInput").ap()
    wpr_d = nc.dram_tensor("w_pr", [P, CL // P, C], F16, kind="ExternalInput").ap()
    bqk_d = nc.dram_tensor("b_qk", [P, 4], F32, kind="ExternalInput").ap()
    qkw_d = nc.dram_tensor("qkw", [P, 4], F32, kind="ExternalInput").ap()
    outT_d = nc.dram_tensor("outT", [C, n], F16, kind="ExternalOutput").ap()

    with tile.TileContext(nc) as tc, ExitStack() as ctx:
        con = ctx.enter_context(tc.tile_pool(name="con", bufs=1))
        wp = ctx.enter_context(tc.tile_pool(name="wp", bufs=1))
        qk = ctx.enter_context(tc.tile_pool(name="qk", bufs=1))
        vp = ctx.enter_context(tc.tile_pool(name="vp", bufs=1))
        xp = ctx.enter_context(tc.tile_pool(name="xp", bufs=1))
        sqp = ctx.enter_context(tc.tile_pool(name="sqp", bufs=4))
        rp = ctx.enter_context(tc.tile_pool(name="rp", bufs=2))
        ptp = ctx.enter_context(tc.tile_pool(name="ptp", bufs=1))
        atp = ctx.enter_context(tc.tile_pool(name="atp", bufs=1))
        rp2 = ctx.enter_context(tc.tile_pool(name="rp2", bufs=2))
        osp = ctx.enter_context(tc.tile_pool(name="osp", bufs=4))
        ps = ctx.enter_context(tc.tile_pool(name="ps", bufs=1, space="PSUM"))

        bqk_sb = con.tile([P, 4], F32, tag="bqk")
        qkw_sb = con.tile([P, 4], F32, tag="qkw")

        # ---- weight tiles ----
        wv_sb = wp.tile([P, kt, CL], F16, tag="wv", name="wv")
        wqk_sb = wp.tile([P, kt, 2 * CL], F16, tag="wqk", name="wqk")
        wpr_sb = wp.tile([P, CL // P, C], F16, tag="wpr", name="wpr")

        # rotating x tiles: 2 j-blocks in flight
        def x_tile(j):
            return xp.tile([P, kt, nq], F16, tag="xt", bufs=2, name=f"xt{j}")

        # persistent attention operands
        qkT = [qk.tile([P, n], F16, tag=f"qkT{m}", name=f"qkT{m}") for m in range(4)]
        v_aug = [vp.tile([P, HL, VW], F16, tag=f"va{i}", name=f"va{i}") for i in range(nb)]
        attnT = [atp.tile([P, n], F16, tag=f"at{t}", name=f"at{t}") for t in range(HL // HPB)]

        # startup: DMAs first so nothing delays the sync queue; v weights
        # and the first half of x(j=0) lead (the first v blocks need them)
        xs = {}
        t = x_tile(0)
        xs[0] = t
        nc.sync.dma_start(wv_sb[:], wv_d[:])
        nc.sync.dma_start(t[:, :, 0:nq // 2], xT_d[:, :, 0:nq // 2])
        nc.sync.dma_start(t[:, :, nq // 2:nq], xT_d[:, :, nq // 2:nq])
        nc.sync.dma_start(bqk_sb[:], bqk_d[:])
        nc.sync.dma_start(qkw_sb[:], qkw_d[:])
        nc.sync.dma_start(wqk_sb[:], wqk_d[:])
        nc.sync.dma_start(wpr_sb[:], wpr_d[:])

        # ---- constants (compute engines; no DMA) ----
        ones_m = con.tile([P, P], F16, tag="onesm")      # lhsT for sumsq
        nc.vector.memset(ones_m[:], 1.0)
        eps_sb = con.tile([P, 1], F32, tag="eps")
        nc.vector.memset(eps_sb[:], EPS)
        # ones columns of v_aug are written once here; the per-block drains
        # only touch the v columns (strided copy)
        ones_c = con.tile([P, HL, 1], F16, tag="onesc")
        nc.gpsimd.memset(ones_c[:], 1.0)
        for i in range(nb):
            nc.gpsimd.tensor_copy(v_aug[i][:, :, D:VW], ones_c[:])

        # ---- stage 1+2, pipelined per token block j ----
        for j in range(nj):
            js = slice(j * nq, (j + 1) * nq)
            if j + 1 < nj:
                t = x_tile(j + 1)
                nc.sync.dma_start(
                    t[:], xT_d[:, :, (j + 1) * nq:(j + 2) * nq])
                xs[j + 1] = t

            # v for this block's nq//P key blocks (b_v is folded into the
            # host-side output bias since attention rows sum to 1); the
            # strided drain leaves the ones columns intact
            for i in range(j * (nq // P), (j + 1) * (nq // P)):
                ioff = i * P - j * nq
                acc = ps.tile([P, HL, D], F32, tag="s2", bufs=2, name="vacc")
                for k in range(kt):
                    nc.tensor.matmul(
                        acc[:], xs[j][:, k:k + 1, ioff:ioff + P],
                        wv_sb[:, k:k + 1, :],
                        start=(k == 0), stop=(k == kt - 1))
                nc.vector.tensor_copy(v_aug[i][:, :, 0:D], acc[:])

            # qk projection + rmsnorm, k-heads (m=2,3) first: attention's
            # kT dependency clears while the q half of the last block
            # computes
            def emit_qk(m):
                # bias-add + PSUM drain on DVE: the scalar engine carries
                # the square/ln/exp rms chain and must stay off the
                # stage-1 critical path
                acc = ps.tile([P, nq], F32, tag="s2", bufs=2, name="qacc")
                for k in range(kt):
                    nc.tensor.matmul(
                        acc[:], wqk_sb[:, k:k + 1, m * P:(m + 1) * P],
                        xs[j][:, k:k + 1, :], start=(k == 0),
                        stop=(k == kt - 1))
                nc.vector.tensor_scalar_add(
                    qkT[m][:, js], acc[:], bqk_sb[:, m:m + 1])

            def emit_rms_for(jr):
                jrs = slice(jr * nq, (jr + 1) * nq)

                def emit_rms(m):
                    # both heads of the m-slice share one ssq tile: the
                    # h2=1 sumsq matmul is tile-positioned at partition 64.
                    # 1/sqrt(ms) is computed as exp(-0.5*ln(ms)): square,
                    # ln, exp, identity and copy all live in one act table
                    # set, so the whole kernel runs without a single
                    # ACT_TABLE_LOAD (sqrt would thrash against the
                    # attention exps at the phase boundary).
                    sq = sqp.tile([P, nq], F16, tag="sq", name="sq")
                    nc.scalar.square(sq[:], qkT[m][:, jrs])
                    ssq = ps.tile([P, nq], F32, tag="oas", bufs=4, name="ssq")
                    for h2 in range(HPB):
                        pr = slice(h2 * D, (h2 + 1) * D)
                        nc.tensor.matmul(ssq[pr, :], ones_m[pr, 0:D],
                                         sq[pr, :], start=True, stop=True)
                    lnm = rp.tile([P, nq], F32, tag="lnm", bufs=4, name="lnm")
                    nc.scalar.activation(lnm[:], ssq[:], AF.Ln,
                                         scale=1.0 / D, bias=eps_sb[:, 0:1])
                    rec = rp.tile([P, nq], F16, tag="rec", bufs=4, name="rec")
                    nc.scalar.activation(rec[:], lnm[:], AF.Exp, scale=-0.5)
                    nc.vector.scalar_tensor_tensor(
                        qkT[m][:, jrs], qkT[m][:, jrs], qkw_sb[:, m:m + 1],
                        rec[:], op0=mybir.AluOpType.mult,
                        op1=mybir.AluOpType.mult)

                return emit_rms

            if j < nj - 1:
                # interleave the deferred rms chains between the qk
                # projections so the ssq matmuls never park the PE behind
                # scalar-engine work that is queued after later qk drains
                emit_qk(2)
                emit_qk(3)
                if j > 0:
                    emit_rms_for(j - 1)(2)
                    emit_rms_for(j - 1)(3)
                emit_qk(0)
                emit_qk(1)
                if j > 0:
                    emit_rms_for(j - 1)(0)
                    emit_rms_for(j - 1)(1)
            else:
                # last block: norm the k heads before the q projections so
                # the attention's kT dependency clears under the q matmuls
                emit_qk(2)
                emit_qk(3)
                if j > 0:
                    emit_rms_for(j - 1)(2)
                    emit_rms_for(j - 1)(3)
                emit_rms_for(j)(2)
                emit_rms_for(j)(3)
                emit_qk(0)
                emit_qk(1)
                if j > 0:
                    emit_rms_for(j - 1)(0)
                    emit_rms_for(j - 1)(1)

        emit_rms_for(nj - 1)(0)
        emit_rms_for(nj - 1)(1)

        # ---- attention: S(cur) || AV(prev), then norm(prev2) + proj ----
        units = [(j, hp) for j in range(nj) for hp in range(HL // HPB)]
        BLK = min(4, nb)

        def emit_s(u, i):
            j, hp = u
            js = slice(j * nq, (j + 1) * nq)
            qm, km = hp, 2 + hp
            s2 = ps.tile([P, 2 * nq], F32, tag="s2", bufs=2, name="s2")
            for sub in range(HPB):
                pr = slice(sub * D, (sub + 1) * D)
                nc.tensor.matmul(
                    s2[:, sub * nq:(sub + 1) * nq],
                    qkT[km][pr, i * P:(i + 1) * P], qkT[qm][pr, js],
                    start=True, stop=True)
            pt = ptp.tile([P, 2 * nq], F16, tag="pt", bufs=20, name="pt")
            nc.scalar.activation(pt[:], s2[:], AF.Exp, scale=SCALE)
            return pt

        def emit_av(u, oas, pts, i):
            j, hp = u
            for sub in range(HPB):
                h = hp * HPB + sub
                nc.tensor.matmul(
                    oas[sub][0:VW, :], v_aug[i][:, h:h + 1, :],
                    pts[i][:, sub * nq:(sub + 1) * nq],
                    start=(i == 0), stop=(i == nb - 1))

        BCAST0 = [0] * 32

        def emit_norm(u, oas, c0=0, c1=nq):
            # Softmax denominators ride in row 64 of each AV output.
            # stream_shuffle broadcasts partition 64 across two 32-row
            # quadrants (it shuffles within 32-partition windows, so two
            # ops), then reciprocal into SBUF and normalize into attnT.
            j, hp = u
            js = slice(j * nq + c0, j * nq + c1)
            for sub in range(HPB):
                bcs = rp2.tile([D, nq], F32, tag=f"bcs{sub}", name="bcs")
                nc.vector.stream_shuffle(bcs[0:32, c0:c1],
                                         oas[sub][D:D + 32, c0:c1], BCAST0)
                nc.vector.stream_shuffle(bcs[32:64, c0:c1],
                                         oas[sub][D:D + 32, c0:c1], BCAST0)
                recn = rp2.tile([D, nq], F32, tag=f"recn{sub}", name="recn")
                nc.vector.reciprocal_approx_fast(recn[:, c0:c1],
                                                 bcs[:, c0:c1])
                pr = slice(sub * D, (sub + 1) * D)
                nc.vector.tensor_mul(attnT[hp][pr, js],
                                     oas[sub][0:D, c0:c1], recn[:, c0:c1])

        def emit_proj_m(j, m, c0=0, c1=nq, drain=None):
            # b_proj (and the folded b_v term) are added on the host, so
            # the drain is a plain PSUM->fp16 copy
            js = slice(j * nq + c0, j * nq + c1)
            acc = ps.tile([P, c1 - c0], F32, tag="s2", bufs=2, name="pacc")
            for t in range(CL // P):
                nc.tensor.matmul(
                    acc[:], wpr_sb[:, t:t + 1, m * P:(m + 1) * P],
                    attnT[t][:, js], start=(t == 0), stop=(t == CL // P - 1))
            ost = osp.tile([P, c1 - c0], F16, tag="ost", name="ost")
            if drain == "act":
                nc.scalar.copy(ost[:], acc[:])
            else:
                nc.vector.tensor_copy(ost[:], acc[:])
            nc.sync.dma_start(outT_d[m * P:(m + 1) * P, js], ost[:])

        nchunks = (nb + BLK - 1) // BLK
        mpc = (C // P) // nchunks      # proj m-steps per chunk
        prev = None    # (unit, oas, pts)
        prev2 = None
        for idx in range(len(units) + 1):
            cur = units[idx] if idx < len(units) else None
            # norm of prev2 first: its reads gate the slot reuse of this
            # iteration's AV writes, so it must precede them in scheduler
            # priority (emitting it later deadlocks the in-order PE queue)
            proj_j = None
            if prev2 is not None:
                emit_norm(prev2[0], prev2[1])
                j2, hp2 = prev2[0]
                if hp2 == HL // HPB - 1:
                    proj_j = j2
            oas_prev = None
            if prev is not None:
                oas_prev = [ps.tile([P, nq], F32, tag="oas", bufs=4,
                                    name=f"oa{s_}") for s_ in range(HPB)]
            pts = {}
            for ib in range(nchunks):
                blk = range(ib * BLK, min((ib + 1) * BLK, nb))
                if cur is not None:
                    for i in blk:
                        pts[i] = emit_s(cur, i)
                if prev is not None:
                    for i in blk:
                        emit_av(prev[0], oas_prev, prev[2], i)
                # proj of the finished block rides along, a couple of
                # m-slices per chunk, so its PSUM slot rotation (and the
                # drain DVE work) spreads across the unit instead of
                # stalling the PE in one burst
                if proj_j is not None:
                    for m in range(ib * mpc, (ib + 1) * mpc):
                        emit_proj_m(proj_j, m)
            if proj_j is not None:
                for m in range(nchunks * mpc, C // P):
                    emit_proj_m(proj_j, m)
            prev2 = (prev[0], oas_prev) if prev is not None else None
            prev = (cur, None, pts) if cur is not None else None

        # tail: the last unit's norm has nothing left to hide under, so
        # run it in column halves with the final projection interleaved;
        # the drains go to the (now idle) scalar engine so they don't
        # serialize behind the norm chain on DVE
        for c0, c1 in ((0, nq // 2), (nq // 2, nq)):
            emit_norm(prev2[0], prev2[1], c0, c1)
            for m in range(C // P):
                emit_proj_m(nj - 1, m, c0, c1, drain="act")

    nc.compile()
    return nc


_NC_CACHE = {}


def _get_nc(n=N, nq=NQ):
    key = (n, nq)
    if key not in _NC_CACHE:
        _NC_CACHE[key] = build(n, nq)
    return _NC_CACHE[key]


def make_in_maps(x, w_qkv, b_qkv, q_w, k_w, w_proj, b_proj):
    """Shard full inputs into per-core in_maps (host side)."""
    def ktile(a):
        # [C, W] -> [P, C//P, W] so one DMA fills the whole SBUF tile
        return np.ascontiguousarray(
            a.reshape(a.shape[0] // P, P, a.shape[1]).transpose(1, 0, 2))

    in_maps = []
    for cid in range(NCORES):
        b, g = cid // GROUPS, cid % GROUPS
        c0 = g * CL
        xT = np.ascontiguousarray(x[b].T)
        w_qk = np.concatenate([w_qkv[:, c0:c0 + CL],
                               w_qkv[:, C + c0:C + c0 + CL]], axis=1)
        w_v = w_qkv[:, 2 * C + c0:2 * C + c0 + CL]
        w_pr = w_proj[c0:c0 + CL, :]
        b_qk = np.stack([b_qkv[c0 + m * P:c0 + (m + 1) * P] for m in range(2)]
                        + [b_qkv[C + c0 + m * P:C + c0 + (m + 1) * P]
                           for m in range(2)], axis=1)
        qkw = np.stack([np.tile(q_w, HPB), np.tile(q_w, HPB),
                        np.tile(k_w, HPB), np.tile(k_w, HPB)], axis=1)
        in_maps.append({
            "xT": ktile(xT).astype(np.float16),
            "w_qk": ktile(w_qk).astype(np.float16),
            "w_v": ktile(w_v).astype(np.float16),
            "w_pr": ktile(w_pr).astype(np.float16),
            "b_qk": np.ascontiguousarray(b_qk).astype(np.float32),
            "qkw": np.ascontiguousarray(qkw).astype(np.float32),
        })
    return in_maps


def kernel(x, w_qkv, b_qkv, q_w, k_w, w_proj, b_proj, _trace=False):
    x = np.asarray(x)
    n = x.shape[1]
    nc = _get_nc(n, NQ if n % NQ == 0 else P)
    in_maps = make_in_maps(np.asarray(x, np.float32), np.asarray(w_qkv, np.float32),
                           np.asarray(b_qkv, np.float32), np.asarray(q_w, np.float32),
                           np.asarray(k_w, np.float32), np.asarray(w_proj, np.float32),
                           np.asarray(b_proj, np.float32))
    res = run_bass_kernel_spmd(nc, in_maps, core_ids=list(range(NCORES)),
                               trace=_trace)
    # TP unshard: sum the 4 head-group fp16 partials per batch, transpose,
    # and add the host-folded bias (b_proj plus b_v @ w_proj: attention
    # rows sum to 1, so the v bias is additive on attn_out)
    bias = (np.asarray(b_proj, np.float32)
            + np.asarray(b_qkv[2 * C:3 * C], np.float32)
            @ np.asarray(w_proj, np.float32))
    out = np.stack([
        sum(res.results[b * GROUPS + g]["outT"].astype(np.float32)
            for g in range(GROUPS)).T + bias
        for b in range(B)
    ]).astype(np.float32)
    if _trace:
        return out, res
    return out


# revision 35
# speedup vs baseline: 1.0946x; 1.0946x over previous
"""Multi-head attention (B=2, N=2048, C=1024, H=16, qk-RMSNorm) on 8 TRN2 cores.

v3 of the kernel. Same sharding as baseline (TP over 4 head groups x DP
over batch; host sums the 4 w_proj partials per batch) and the same
attention pipeline (S(cur) || AV(prev), then norm(prev2) + proj).

Changes vs v2 (299.98us):
- fp16 everywhere 16-bit data goes: x, w_qkv, w_proj, qkT, pt, v_aug,
  attnT, sums. Same PE rate as fp32r/bf16, half the DMA + LDWEIGHTS
  bytes of the fp32r GEMM weights, and 4x less quantization noise than
  bf16 (fp16 has 10 mantissa bits vs bf16's 8; all magnitudes here are
  well inside fp16 range: |logits*scale| < ~6 -> pt < ~450 < 65504).
- v_aug is produced directly by the v GEMM: w_v is host-padded to
  [C, HL*65] with zero columns at the per-head ones positions and the
  bias row carries 1.0 there, so acc = x@wv_aug + ones*bv_aug lands in
  the augmented layout. One DVE copy per key block replaces the 4
  scalar.copy + 4 gpsimd ones-writes of v2 (~30us of ACT/Pool work).
- RMSNorm: the two heads of an m-slice share one ssq PSUM tile (the
  second head's sumsq matmul is tile-positioned at partition 64), so
  sqrt / reciprocal / scale-mul run once per m-slice on [128, nq]
  instead of per head on [64, nq]: halves those ACT/DVE instr counts.
  sq is computed from the fp16 qkT in SBUF (2x DVE mode) instead of
  from PSUM.

PSUM budget unchanged: tag "s2" [128,1024] bufs=2, tag "oas" [128,512]
bufs=4.
"""

import sys

if "/opt/trn_rl_repo" not in sys.path:
    sys.path.insert(0, "/opt/trn_rl_repo")

from contextlib import ExitStack

import numpy as np

import concourse.mybir as mybir
import concourse.tile as tile
from concourse import bacc
from concourse.bass_utils import run_bass_kernel_spmd

F32 = mybir.dt.float32
F16 = mybir.dt.float16
AF = mybir.ActivationFunctionType

B, N, C, H = 2, 2048, 1024, 16
D = C // H          # 64
EPS = 1e-6
NCORES = 8
GROUPS = 4          # head groups (cores per batch)
HL = H // GROUPS    # heads per core = 4
CL = HL * D         # local channels = 256
SCALE = D ** -0.5   # 0.125

P = 128             # partition dim
KT = C // P         # 8 contraction tiles over C
NQ = 512            # query/token block
HPB = P // D        # heads per 128-channel block = 2
VW = D + 1          # 65: v columns + ones column
CLA = HL * VW       # 260: augmented v width


def build(n=N, nq=NQ):
    nb = n // P          # key blocks of 128
    nj = n // nq         # token blocks of nq
    kt = KT

    nc = bacc.Bacc("TRN2", target_bir_lowering=False, debug=False,
                   num_devices=NCORES)

    # inputs are host-pretiled so each loads with a single wide DMA
    # (DMA triggers serialize at ~600ns apiece on the sync queue)
    xT_d = nc.dram_tensor("xT", [P, KT, n], F16, kind="ExternalInput").ap()
    wqk_d = nc.dram_tensor("w_qk", [P, KT, 2 * CL], F16, kind="ExternalInput").ap()
    wv_d = nc.dram_tensor("w_v", [P, KT, CL], F16, kind="ExternalInput").ap()
    wpr_d = nc.dram_tensor("w_pr", [P, CL // P, C], F16, kind="ExternalInput").ap()
    bqk_d = nc.dram_tensor("b_qk", [P, 4], F32, kind="ExternalInput").ap()
    qkw_d = nc.dram_tensor("qkw", [P, 4], F32, kind="ExternalInput").ap()
    outT_d = nc.dram_tensor("outT", [C, n], F16, kind="ExternalOutput").ap()

    # preload the one act-table set that serves every activation in this
    # kernel (exp, ln, square, identity, copy); without this the table-load
    # pass ping-pongs between per-function sets (~20 loads x 1.3us)
    from concourse.hw_specs import get_activation_tables
    _tables = list(get_activation_tables(nc.m.arch).keys())
    _set_id = _tables.index("natural_log_exp_and_others")

    with tile.TileContext(nc) as tc, ExitStack() as ctx:
        nc.scalar.add_instruction(mybir.InstLoadActFuncSet(
            name=nc.get_next_instruction_name(),
            ins=[], outs=[], act_func_set_id=_set_id))
        con = ctx.enter_context(tc.tile_pool(name="con", bufs=1))
        wp = ctx.enter_context(tc.tile_pool(name="wp", bufs=1))
        qk = ctx.enter_context(tc.tile_pool(name="qk", bufs=1))
        vp = ctx.enter_context(tc.tile_pool(name="vp", bufs=1))
        xp = ctx.enter_context(tc.tile_pool(name="xp", bufs=1))
        sqp = ctx.enter_context(tc.tile_pool(name="sqp", bufs=4))
        rp = ctx.enter_context(tc.tile_pool(name="rp", bufs=2))
        ptp = ctx.enter_context(tc.tile_pool(name="ptp", bufs=1))
        atp = ctx.enter_context(tc.tile_pool(name="atp", bufs=1))
        rp2 = ctx.enter_context(tc.tile_pool(name="rp2", bufs=2))
        osp = ctx.enter_context(tc.tile_pool(name="osp", bufs=4))
        ps = ctx.enter_context(tc.tile_pool(name="ps", bufs=1, space="PSUM"))

        bqk_sb = con.tile([P, 4], F32, tag="bqk")
        qkw_sb = con.tile([P, 4], F32, tag="qkw")

        # ---- weight tiles ----
        wv_sb = wp.tile([P, kt, CL], F16, tag="wv", name="wv")
        wqk_sb = wp.tile([P, kt, 2 * CL], F16, tag="wqk", name="wqk")
        wpr_sb = wp.tile([P, CL // P, C], F16, tag="wpr", name="wpr")

        # rotating x tiles: 2 j-blocks in flight
        def x_tile(j):
            return xp.tile([P, kt, nq], F16, tag="xt", bufs=2, name=f"xt{j}")

        # persistent attention operands
        qkT = [qk.tile([P, n], F16, tag=f"qkT{m}", name=f"qkT{m}") for m in range(4)]
        v_aug = [vp.tile([P, HL, VW], F16, tag=f"va{i}", name=f"va{i}") for i in range(nb)]
        attnT = [atp.tile([P, n], F16, tag=f"at{t}", name=f"at{t}") for t in range(HL // HPB)]

        # startup: DMAs first so nothing delays the sync queue; v weights
        # and the first half of x(j=0) lead (the first v blocks need them)
        xs = {}
        t = x_tile(0)
        xs[0] = t
        nc.sync.dma_start(wv_sb[:], wv_d[:])
        nc.sync.dma_start(t[:, :, 0:nq // 2], xT_d[:, :, 0:nq // 2])
        nc.sync.dma_start(t[:, :, nq // 2:nq], xT_d[:, :, nq // 2:nq])
        nc.sync.dma_start(bqk_sb[:], bqk_d[:])
        nc.sync.dma_start(qkw_sb[:], qkw_d[:])
        nc.sync.dma_start(wqk_sb[:], wqk_d[:])
        nc.sync.dma_start(wpr_sb[:], wpr_d[:])

        # ---- constants (compute engines; no DMA) ----
        ones_m = con.tile([P, P], F16, tag="onesm")      # lhsT for sumsq
        nc.vector.memset(ones_m[:], 1.0)
        eps_sb = con.tile([P, 1], F32, tag="eps")
        nc.vector.memset(eps_sb[:], EPS)
        # ones columns of v_aug are written once here; the per-block drains
        # only touch the v columns (strided copy)
        ones_c = con.tile([P, HL, 1], F16, tag="onesc")
        nc.gpsimd.memset(ones_c[:], 1.0)
        for i in range(nb):
            nc.gpsimd.tensor_copy(v_aug[i][:, :, D:VW], ones_c[:])

        # ---- stage 1+2, pipelined per token block j ----
        for j in range(nj):
            js = slice(j * nq, (j + 1) * nq)
            if j + 1 < nj:
                t = x_tile(j + 1)
                nc.sync.dma_start(
                    t[:], xT_d[:, :, (j + 1) * nq:(j + 2) * nq])
                xs[j + 1] = t

            # v for this block's nq//P key blocks (b_v is folded into the
            # host-side output bias since attention rows sum to 1); the
            # strided drain leaves the ones columns intact
            for i in range(j * (nq // P), (j + 1) * (nq // P)):
                ioff = i * P - j * nq
                acc = ps.tile([P, HL, D], F32, tag="s2", bufs=2, name="vacc")
                for k in range(kt):
                    nc.tensor.matmul(
                        acc[:], xs[j][:, k:k + 1, ioff:ioff + P],
                        wv_sb[:, k:k + 1, :],
                        start=(k == 0), stop=(k == kt - 1))
                nc.vector.tensor_copy(v_aug[i][:, :, 0:D], acc[:])

            # qk projection + rmsnorm, k-heads (m=2,3) first: attention's
            # kT dependency clears while the q half of the last block
            # computes
            def emit_qk(m):
                # bias-add + PSUM drain on DVE: the scalar engine carries
                # the square/ln/exp rms chain and must stay off the
                # stage-1 critical path
                acc = ps.tile([P, nq], F32, tag="s2", bufs=2, name="qacc")
                for k in range(kt):
                    nc.tensor.matmul(
                        acc[:], wqk_sb[:, k:k + 1, m * P:(m + 1) * P],
                        xs[j][:, k:k + 1, :], start=(k == 0),
                        stop=(k == kt - 1))
                nc.vector.tensor_scalar_add(
                    qkT[m][:, js], acc[:], bqk_sb[:, m:m + 1])

            def emit_rms_for(jr):
                jrs = slice(jr * nq, (jr + 1) * nq)

                def emit_rms(m):
                    # both heads of the m-slice share one ssq tile: the
                    # h2=1 sumsq matmul is tile-positioned at partition 64.
                    # 1/sqrt(ms) is computed as exp(-0.5*ln(ms)): square,
                    # ln, exp, identity and copy all live in one act table
                    # set, so the whole kernel runs without a single
                    # ACT_TABLE_LOAD (sqrt would thrash against the
                    # attention exps at the phase boundary).
                    sq = sqp.tile([P, nq], F16, tag="sq", name="sq")
                    nc.scalar.square(sq[:], qkT[m][:, jrs])
                    ssq = ps.tile([P, nq], F32, tag="oas", bufs=4, name="ssq")
                    for h2 in range(HPB):
                        pr = slice(h2 * D, (h2 + 1) * D)
                        nc.tensor.matmul(ssq[pr, :], ones_m[pr, 0:D],
                                         sq[pr, :], start=True, stop=True)
                    lnm = rp.tile([P, nq], F32, tag="lnm", bufs=4, name="lnm")
                    nc.scalar.activation(lnm[:], ssq[:], AF.Ln,
                                         scale=1.0 / D, bias=eps_sb[:, 0:1])
                    rec = rp.tile([P, nq], F16, tag="rec", bufs=4, name="rec")
                    nc.scalar.activation(rec[:], lnm[:], AF.Exp, scale=-0.5)
                    nc.vector.scalar_tensor_tensor(
                        qkT[m][:, jrs], qkT[m][:, jrs], qkw_sb[:, m:m + 1],
                        rec[:], op0=mybir.AluOpType.mult,
                        op1=mybir.AluOpType.mult)

                return emit_rms

            if j < nj - 1:
                # interleave the deferred rms chains between the qk
                # projections so the ssq matmuls never park the PE behind
                # scalar-engine work that is queued after later qk drains
                emit_qk(2)
                emit_qk(3)
                if j > 0:
                    emit_rms_for(j - 1)(2)
                    emit_rms_for(j - 1)(3)
                emit_qk(0)
                emit_qk(1)
                if j > 0:
                    emit_rms_for(j - 1)(0)
                    emit_rms_for(j - 1)(1)
            else:
                # last block: norm the k heads before the q projections so
                # the attention's kT dependency clears under the q matmuls
                emit_qk(2)
                emit_qk(3)
                if j > 0:
                    emit_rms_for(j - 1)(2)
                    emit_rms_for(j - 1)(3)
                emit_rms_for(j)(2)
                emit_rms_for(j)(3)
                emit_qk(0)
                emit_qk(1)
                if j > 0:
                    emit_rms_for(j - 1)(0)
                    emit_rms_for(j - 1)(1)

        emit_rms_for(nj - 1)(0)
        emit_rms_for(nj - 1)(1)

        # ---- attention: S(cur) || AV(prev), then norm(prev2) + proj ----
        units = [(j, hp) for j in range(nj) for hp in range(HL // HPB)]
        BLK = min(4, nb)

        def emit_s(u, i):
            j, hp = u
            js = slice(j * nq, (j + 1) * nq)
            qm, km = hp, 2 + hp
            s2 = ps.tile([P, 2 * nq], F32, tag="s2", bufs=2, name="s2")
            for sub in range(HPB):
                pr = slice(sub * D, (sub + 1) * D)
                nc.tensor.matmul(
                    s2[:, sub * nq:(sub + 1) * nq],
                    qkT[km][pr, i * P:(i + 1) * P], qkT[qm][pr, js],
                    start=True, stop=True)
            pt = ptp.tile([P, 2 * nq], F16, tag="pt", bufs=20, name="pt")
            nc.scalar.activation(pt[:], s2[:], AF.Exp, scale=SCALE)
            return pt

        def emit_av(u, oas, pts, i):
            j, hp = u
            for sub in range(HPB):
                h = hp * HPB + sub
                nc.tensor.matmul(
                    oas[sub][0:VW, :], v_aug[i][:, h:h + 1, :],
                    pts[i][:, sub * nq:(sub + 1) * nq],
                    start=(i == 0), stop=(i == nb - 1))

        BCAST0 = [0] * 32

        def emit_norm(u, oas, c0=0, c1=nq):
            # Softmax denominators ride in row 64 of each AV output.
            # stream_shuffle broadcasts partition 64 across two 32-row
            # quadrants (it shuffles within 32-partition windows, so two
            # ops), then reciprocal into SBUF and normalize into attnT.
            j, hp = u
            js = slice(j * nq + c0, j * nq + c1)
            for sub in range(HPB):
                bcs = rp2.tile([D, nq], F32, tag=f"bcs{sub}", name="bcs")
                nc.vector.stream_shuffle(bcs[0:32, c0:c1],
                                         oas[sub][D:D + 32, c0:c1], BCAST0)
                nc.vector.stream_shuffle(bcs[32:64, c0:c1],
                                         oas[sub][D:D + 32, c0:c1], BCAST0)
                recn = rp2.tile([D, nq], F32, tag=f"recn{sub}", name="recn")
                nc.vector.reciprocal_approx_fast(recn[:, c0:c1],
                                                 bcs[:, c0:c1])
                pr = slice(sub * D, (sub + 1) * D)
                nc.vector.tensor_mul(attnT[hp][pr, js],
                                     oas[sub][0:D, c0:c1], recn[:, c0:c1])

        def emit_proj_m(j, m, c0=0, c1=nq, drain=None):
            # b_proj (and the folded b_v term) are added on the host, so
            # the drain is a plain PSUM->fp16 copy
            js = slice(j * nq + c0, j * nq + c1)
            acc = ps.tile([P, c1 - c0], F32, tag="s2", bufs=2, name="pacc")
            for t in range(CL // P):
                nc.tensor.matmul(
                    acc[:], wpr_sb[:, t:t + 1, m * P:(m + 1) * P],
                    attnT[t][:, js], start=(t == 0), stop=(t == CL // P - 1))
            ost = osp.tile([P, c1 - c0], F16, tag="ost", name="ost")
            if drain == "act":
                nc.scalar.copy(ost[:], acc[:])
            else:
                nc.vector.tensor_copy(ost[:], acc[:])
            nc.sync.dma_start(outT_d[m * P:(m + 1) * P, js], ost[:])

        nchunks = (nb + BLK - 1) // BLK
        mpc = (C // P) // nchunks      # proj m-steps per chunk
        prev = None    # (unit, oas, pts)
        prev2 = None
        for idx in range(len(units) + 1):
            cur = units[idx] if idx < len(units) else None
            # norm of prev2 first: its reads gate the slot reuse of this
            # iteration's AV writes, so it must precede them in scheduler
            # priority (emitting it later deadlocks the in-order PE queue)
            proj_j = None
            if prev2 is not None:
                emit_norm(prev2[0], prev2[1])
                j2, hp2 = prev2[0]
                if hp2 == HL // HPB - 1:
                    proj_j = j2
            oas_prev = None
            if prev is not None:
                oas_prev = [ps.tile([P, nq], F32, tag="oas", bufs=4,
                                    name=f"oa{s_}") for s_ in range(HPB)]
            pts = {}
            for ib in range(nchunks):
                blk = range(ib * BLK, min((ib + 1) * BLK, nb))
                if cur is not None:
                    for i in blk:
                        pts[i] = emit_s(cur, i)
                if prev is not None:
                    for i in blk:
                        emit_av(prev[0], oas_prev, prev[2], i)
                # proj of the finished block rides along, a couple of
                # m-slices per chunk, so its PSUM slot rotation (and the
                # drain DVE work) spreads across the unit instead of
                # stalling the PE in one burst
                if proj_j is not None:
                    for m in range(ib * mpc, (ib + 1) * mpc):
                        emit_proj_m(proj_j, m)
            if proj_j is not None:
                for m in range(nchunks * mpc, C // P):
                    emit_proj_m(proj_j, m)
            prev2 = (prev[0], oas_prev) if prev is not None else None
            prev = (cur, None, pts) if cur is not None else None

        # tail: the last unit's norm has nothing left to hide under, so
        # run it in column halves with the final projection interleaved;
        # the drains go to the (now idle) scalar engine so they don't
        # serialize behind the norm chain on DVE
        for c0, c1 in ((0, nq // 2), (nq // 2, nq)):
            emit_norm(prev2[0], prev2[1], c0, c1)
            for m in range(C // P):
                emit_proj_m(nj - 1, m, c0, c1, drain="act")

    nc.compile()
    return nc


_NC_CACHE = {}


def _get_nc(n=N, nq=NQ):
    key = (n, nq)
    if key not in _NC_CACHE:
        _NC_CACHE[key] = build(n, nq)
    return _NC_CACHE[key]


def make_in_maps(x, w_qkv, b_qkv, q_w, k_w, w_proj, b_proj):
    """Shard full inputs into per-core in_maps (host side)."""
    def ktile(a):
        # [C, W] -> [P, C//P, W] so one DMA fills the whole SBUF tile
        return np.ascontiguousarray(
            a.reshape(a.shape[0] // P, P, a.shape[1]).transpose(1, 0, 2))

    in_maps = []
    for cid in range(NCORES):
        b, g = cid // GROUPS, cid % GROUPS
        c0 = g * CL
        xT = np.ascontiguousarray(x[b].T)
        w_qk = np.concatenate([w_qkv[:, c0:c0 + CL],
                               w_qkv[:, C + c0:C + c0 + CL]], axis=1)
        w_v = w_qkv[:, 2 * C + c0:2 * C + c0 + CL]
        w_pr = w_proj[c0:c0 + CL, :]
        b_qk = np.stack([b_qkv[c0 + m * P:c0 + (m + 1) * P] for m in range(2)]
                        + [b_qkv[C + c0 + m * P:C + c0 + (m + 1) * P]
                           for m in range(2)], axis=1)
        qkw = np.stack([np.tile(q_w, HPB), np.tile(q_w, HPB),
                        np.tile(k_w, HPB), np.tile(k_w, HPB)], axis=1)
        in_maps.append({
            "xT": ktile(xT).astype(np.float16),
            "w_qk": ktile(w_qk).astype(np.float16),
            "w_v": ktile(w_v).astype(np.float16),
            "w_pr": ktile(w_pr).astype(np.float16),
            "b_qk": np.ascontiguousarray(b_qk).astype(np.float32),
            "qkw": np.ascontiguousarray(qkw).astype(np.float32),
        })
    return in_maps


def kernel(x, w_qkv, b_qkv, q_w, k_w, w_proj, b_proj, _trace=False):
    x = np.asarray(x)
    n = x.shape[1]
    nc = _get_nc(n, NQ if n % NQ == 0 else P)
    in_maps = make_in_maps(np.asarray(x, np.float32), np.asarray(w_qkv, np.float32),
                           np.asarray(b_qkv, np.float32), np.asarray(q_w, np.float32),
                           np.asarray(k_w, np.float32), np.asarray(w_proj, np.float32),
                           np.asarray(b_proj, np.float32))
    res = run_bass_kernel_spmd(nc, in_maps, core_ids=list(range(NCORES)),
                               trace=_trace)
    # TP unshard: sum the 4 head-group fp16 partials per batch, transpose,
    # and add the host-folded bias (b_proj plus b_v @ w_proj: attention
    # rows sum to 1, so the v bias is additive on attn_out)
    bias = (np.asarray(b_proj, np.float32)
            + np.asarray(b_qkv[2 * C:3 * C], np.float32)
            @ np.asarray(w_proj, np.float32))
    out = np.stack([
        sum(res.results[b * GROUPS + g]["outT"].astype(np.float32)
            for g in range(GROUPS)).T + bias
        for b in range(B)
    ]).astype(np.float32)
    if _trace:
        return out, res
    return out


# revision 41
# speedup vs baseline: 1.0996x; 1.0045x over previous
"""Multi-head attention (B=2, N=2048, C=1024, H=16, qk-RMSNorm) on 8 TRN2 cores.

v3 of the kernel. Same sharding as baseline (TP over 4 head groups x DP
over batch; host sums the 4 w_proj partials per batch) and the same
attention pipeline (S(cur) || AV(prev), then norm(prev2) + proj).

Changes vs v2 (299.98us):
- fp16 everywhere 16-bit data goes: x, w_qkv, w_proj, qkT, pt, v_aug,
  attnT, sums. Same PE rate as fp32r/bf16, half the DMA + LDWEIGHTS
  bytes of the fp32r GEMM weights, and 4x less quantization noise than
  bf16 (fp16 has 10 mantissa bits vs bf16's 8; all magnitudes here are
  well inside fp16 range: |logits*scale| < ~6 -> pt < ~450 < 65504).
- v_aug is produced directly by the v GEMM: w_v is host-padded to
  [C, HL*65] with zero columns at the per-head ones positions and the
  bias row carries 1.0 there, so acc = x@wv_aug + ones*bv_aug lands in
  the augmented layout. One DVE copy per key block replaces the 4
  scalar.copy + 4 gpsimd ones-writes of v2 (~30us of ACT/Pool work).
- RMSNorm: the two heads of an m-slice share one ssq PSUM tile (the
  second head's sumsq matmul is tile-positioned at partition 64), so
  sqrt / reciprocal / scale-mul run once per m-slice on [128, nq]
  instead of per head on [64, nq]: halves those ACT/DVE instr counts.
  sq is computed from the fp16 qkT in SBUF (2x DVE mode) instead of
  from PSUM.

PSUM budget unchanged: tag "s2" [128,1024] bufs=2, tag "oas" [128,512]
bufs=4.
"""

import sys

if "/opt/trn_rl_repo" not in sys.path:
    sys.path.insert(0, "/opt/trn_rl_repo")

from contextlib import ExitStack

import numpy as np

import concourse.mybir as mybir
import concourse.tile as tile
from concourse import bacc
from concourse.bass_utils import run_bass_kernel_spmd

F32 = mybir.dt.float32
F16 = mybir.dt.float16
AF = mybir.ActivationFunctionType

B, N, C, H = 2, 2048, 1024, 16
D = C // H          # 64
EPS = 1e-6
NCORES = 8
GROUPS = 4          # head groups (cores per batch)
HL = H // GROUPS    # heads per core = 4
CL = HL * D         # local channels = 256
SCALE = D ** -0.5   # 0.125

P = 128             # partition dim
KT = C // P         # 8 contraction tiles over C
NQ = 512            # query/token block
HPB = P // D        # heads per 128-channel block = 2
VW = D + 1          # 65: v columns + ones column
CLA = HL * VW       # 260: augmented v width


def build(n=N, nq=NQ):
    nb = n // P          # key blocks of 128
    nj = n // nq         # token blocks of nq
    kt = KT

    nc = bacc.Bacc("TRN2", target_bir_lowering=False, debug=False,
                   num_devices=NCORES)

    # inputs are host-pretiled so each loads with a single wide DMA
    # (DMA triggers serialize at ~600ns apiece on the sync queue)
    xT_d = nc.dram_tensor("xT", [P, KT, n], F16, kind="ExternalInput").ap()
    wqk_d = nc.dram_tensor("w_qk", [P, KT, 2 * CL], F16, kind="ExternalInput").ap()
    wv_d = nc.dram_tensor("w_v", [P, KT, CL], F16, kind="ExternalInput").ap()
    wpr_d = nc.dram_tensor("w_pr", [P, CL // P, C], F16, kind="ExternalInput").ap()
    bqk_d = nc.dram_tensor("b_qk", [P, 4], F32, kind="ExternalInput").ap()
    qkw_d = nc.dram_tensor("qkw", [P, 4], F32, kind="ExternalInput").ap()
    outT_d = nc.dram_tensor("outT", [P, C // P, n], F16,
                            kind="ExternalOutput").ap()

    # preload the one act-table set that serves every activation in this
    # kernel (exp, ln, square, identity, copy); without this the table-load
    # pass ping-pongs between per-function sets (~20 loads x 1.3us)
    from concourse.hw_specs import get_activation_tables
    _tables = list(get_activation_tables(nc.m.arch).keys())
    _set_id = _tables.index("natural_log_exp_and_others")

    with tile.TileContext(nc) as tc, ExitStack() as ctx:
        nc.scalar.add_instruction(mybir.InstLoadActFuncSet(
            name=nc.get_next_instruction_name(),
            ins=[], outs=[], act_func_set_id=_set_id))
        con = ctx.enter_context(tc.tile_pool(name="con", bufs=1))
        wp = ctx.enter_context(tc.tile_pool(name="wp", bufs=1))
        qk = ctx.enter_context(tc.tile_pool(name="qk", bufs=1))
        vp = ctx.enter_context(tc.tile_pool(name="vp", bufs=1))
        xp = ctx.enter_context(tc.tile_pool(name="xp", bufs=1))
        sqp = ctx.enter_context(tc.tile_pool(name="sqp", bufs=4))
        rp = ctx.enter_context(tc.tile_pool(name="rp", bufs=2))
        ptp = ctx.enter_context(tc.tile_pool(name="ptp", bufs=1))
        atp = ctx.enter_context(tc.tile_pool(name="atp", bufs=1))
        rp2 = ctx.enter_context(tc.tile_pool(name="rp2", bufs=2))
        osp = ctx.enter_context(tc.tile_pool(name="osp", bufs=4))
        ps = ctx.enter_context(tc.tile_pool(name="ps", bufs=1, space="PSUM"))

        bqk_sb = con.tile([P, 4], F32, tag="bqk")
        qkw_sb = con.tile([P, 4], F32, tag="qkw")

        # ---- weight tiles ----
        wv_sb = wp.tile([P, kt, CL], F16, tag="wv", name="wv")
        wqk_sb = wp.tile([P, kt, 2 * CL], F16, tag="wqk", name="wqk")
        wpr_sb = wp.tile([P, CL // P, C], F16, tag="wpr", name="wpr")

        # rotating x tiles: 2 j-blocks in flight
        def x_tile(j):
            return xp.tile([P, kt, nq], F16, tag="xt", bufs=2, name=f"xt{j}")

        # persistent attention operands
        qkT = [qk.tile([P, n], F16, tag=f"qkT{m}", name=f"qkT{m}") for m in range(4)]
        v_aug = [vp.tile([P, HL, VW], F16, tag=f"va{i}", name=f"va{i}") for i in range(nb)]
        attnT = [atp.tile([P, n], F16, tag=f"at{t}", name=f"at{t}") for t in range(HL // HPB)]

        # startup: the critical loads (v weights + x(j=0)) are spread
        # across three HWDGE queues (scalar/sync/vector) so their
        # transfers run in parallel - a single dma_start only reaches
        # ~70GB/s. The small-but-descriptor-heavy bias loads trail the
        # weights they are needed with.
        xs = {}
        t = x_tile(0)
        xs[0] = t
        nc.scalar.dma_start(wv_sb[:], wv_d[:])
        nc.sync.dma_start(t[:, :, 0:nq // 2], xT_d[:, :, 0:nq // 2])
        nc.gpsimd.dma_start(t[:, :, nq // 2:nq], xT_d[:, :, nq // 2:nq])
        nc.sync.dma_start(wqk_sb[:], wqk_d[:])
        nc.sync.dma_start(bqk_sb[:], bqk_d[:])
        nc.sync.dma_start(qkw_sb[:], qkw_d[:])
        nc.sync.dma_start(wpr_sb[:], wpr_d[:])

        # ---- constants (compute engines; no DMA) ----
        ones_m = con.tile([P, P], F16, tag="onesm")      # lhsT for sumsq
        nc.vector.memset(ones_m[:], 1.0)
        eps_sb = con.tile([P, 1], F32, tag="eps")
        nc.vector.memset(eps_sb[:], EPS)
        # ones columns of v_aug are written once here; the per-block drains
        # only touch the v columns (strided copy)
        ones_c = con.tile([P, HL, 1], F16, tag="onesc")
        nc.gpsimd.memset(ones_c[:], 1.0)
        for i in range(nb):
            nc.gpsimd.tensor_copy(v_aug[i][:, :, D:VW], ones_c[:])

        # ---- stage 1+2, pipelined per token block j ----
        for j in range(nj):
            js = slice(j * nq, (j + 1) * nq)
            if j + 1 < nj:
                t = x_tile(j + 1)
                nc.sync.dma_start(
                    t[:], xT_d[:, :, (j + 1) * nq:(j + 2) * nq])
                xs[j + 1] = t

            # v for this block's nq//P key blocks (b_v is folded into the
            # host-side output bias since attention rows sum to 1); the
            # strided drain leaves the ones columns intact
            for i in range(j * (nq // P), (j + 1) * (nq // P)):
                ioff = i * P - j * nq
                acc = ps.tile([P, HL, D], F32, tag="s2", bufs=2, name="vacc")
                for k in range(kt):
                    nc.tensor.matmul(
                        acc[:], xs[j][:, k:k + 1, ioff:ioff + P],
                        wv_sb[:, k:k + 1, :],
                        start=(k == 0), stop=(k == kt - 1))
                nc.vector.tensor_copy(v_aug[i][:, :, 0:D], acc[:])

            # qk projection + rmsnorm, k-heads (m=2,3) first: attention's
            # kT dependency clears while the q half of the last block
            # computes
            def emit_qk(m):
                # bias-add + PSUM drain on DVE: the scalar engine carries
                # the square/ln/exp rms chain and must stay off the
                # stage-1 critical path
                acc = ps.tile([P, nq], F32, tag="s2", bufs=2, name="qacc")
                for k in range(kt):
                    nc.tensor.matmul(
                        acc[:], wqk_sb[:, k:k + 1, m * P:(m + 1) * P],
                        xs[j][:, k:k + 1, :], start=(k == 0),
                        stop=(k == kt - 1))
                nc.vector.tensor_scalar_add(
                    qkT[m][:, js], acc[:], bqk_sb[:, m:m + 1])

            def emit_rms_for(jr):
                jrs = slice(jr * nq, (jr + 1) * nq)

                def emit_rms(m):
                    # both heads of the m-slice share one ssq tile: the
                    # h2=1 sumsq matmul is tile-positioned at partition 64.
                    # 1/sqrt(ms) is computed as exp(-0.5*ln(ms)): square,
                    # ln, exp, identity and copy all live in one act table
                    # set, so the whole kernel runs without a single
                    # ACT_TABLE_LOAD (sqrt would thrash against the
                    # attention exps at the phase boundary).
                    sq = sqp.tile([P, nq], F16, tag="sq", name="sq")
                    nc.scalar.square(sq[:], qkT[m][:, jrs])
                    ssq = ps.tile([P, nq], F32, tag="oas", bufs=4, name="ssq")
                    for h2 in range(HPB):
                        pr = slice(h2 * D, (h2 + 1) * D)
                        nc.tensor.matmul(ssq[pr, :], ones_m[pr, 0:D],
                                         sq[pr, :], start=True, stop=True)
                    lnm = rp.tile([P, nq], F32, tag="lnm", bufs=4, name="lnm")
                    nc.scalar.activation(lnm[:], ssq[:], AF.Ln,
                                         scale=1.0 / D, bias=eps_sb[:, 0:1])
                    rec = rp.tile([P, nq], F16, tag="rec", bufs=4, name="rec")
                    nc.scalar.activation(rec[:], lnm[:], AF.Exp, scale=-0.5)
                    nc.vector.scalar_tensor_tensor(
                        qkT[m][:, jrs], qkT[m][:, jrs], qkw_sb[:, m:m + 1],
                        rec[:], op0=mybir.AluOpType.mult,
                        op1=mybir.AluOpType.mult)

                return emit_rms

            if j < nj - 1:
                # interleave the deferred rms chains between the qk
                # projections so the ssq matmuls never park the PE behind
                # scalar-engine work that is queued after later qk drains
                emit_qk(2)
                emit_qk(3)
                if j > 0:
                    emit_rms_for(j - 1)(2)
                    emit_rms_for(j - 1)(3)
                emit_qk(0)
                emit_qk(1)
                if j > 0:
                    emit_rms_for(j - 1)(0)
                    emit_rms_for(j - 1)(1)
            else:
                # last block: norm the k heads before the q projections so
                # the attention's kT dependency clears under the q matmuls
                emit_qk(2)
                emit_qk(3)
                if j > 0:
                    emit_rms_for(j - 1)(2)
                    emit_rms_for(j - 1)(3)
                emit_rms_for(j)(2)
                emit_rms_for(j)(3)
                emit_qk(0)
                emit_qk(1)
                if j > 0:
                    emit_rms_for(j - 1)(0)
                    emit_rms_for(j - 1)(1)

        emit_rms_for(nj - 1)(0)
        emit_rms_for(nj - 1)(1)

        # ---- attention: S(cur) || AV(prev), then norm(prev2) + proj ----
        units = [(j, hp) for j in range(nj) for hp in range(HL // HPB)]
        BLK = min(4, nb)

        def emit_s(u, i):
            j, hp = u
            js = slice(j * nq, (j + 1) * nq)
            qm, km = hp, 2 + hp
            s2 = ps.tile([P, 2 * nq], F32, tag="s2", bufs=2, name="s2")
            for sub in range(HPB):
                pr = slice(sub * D, (sub + 1) * D)
                nc.tensor.matmul(
                    s2[:, sub * nq:(sub + 1) * nq],
                    qkT[km][pr, i * P:(i + 1) * P], qkT[qm][pr, js],
                    start=True, stop=True)
            pt = ptp.tile([P, 2 * nq], F16, tag="pt", bufs=20, name="pt")
            nc.scalar.activation(pt[:], s2[:], AF.Exp, scale=SCALE)
            return pt

        def emit_av(u, oas, pts, i):
            j, hp = u
            for sub in range(HPB):
                h = hp * HPB + sub
                nc.tensor.matmul(
                    oas[sub][0:VW, :], v_aug[i][:, h:h + 1, :],
                    pts[i][:, sub * nq:(sub + 1) * nq],
                    start=(i == 0), stop=(i == nb - 1))

        BCAST0 = [0] * 32

        def emit_norm(u, oas, c0=0, c1=nq):
            # Softmax denominators ride in row 64 of each AV output.
            # stream_shuffle broadcasts partition 64 across two 32-row
            # quadrants (it shuffles within 32-partition windows, so two
            # ops), then reciprocal into SBUF and normalize into attnT.
            j, hp = u
            js = slice(j * nq + c0, j * nq + c1)
            for sub in range(HPB):
                bcs = rp2.tile([D, nq], F32, tag=f"bcs{sub}", name="bcs")
                nc.vector.stream_shuffle(bcs[0:32, c0:c1],
                                         oas[sub][D:D + 32, c0:c1], BCAST0)
                nc.vector.stream_shuffle(bcs[32:64, c0:c1],
                                         oas[sub][D:D + 32, c0:c1], BCAST0)
                recn = rp2.tile([D, nq], F32, tag=f"recn{sub}", name="recn")
                nc.vector.reciprocal_approx_fast(recn[:, c0:c1],
                                                 bcs[:, c0:c1])
                pr = slice(sub * D, (sub + 1) * D)
                nc.vector.tensor_mul(attnT[hp][pr, js],
                                     oas[sub][0:D, c0:c1], recn[:, c0:c1])

        def emit_proj_m(j, m, c0=0, c1=nq, drain=None, gather=None):
            # b_proj (and the folded b_v term) are added on the host, so
            # the drain is a plain PSUM->fp16 copy. `gather` batches the
            # drains of several m-slices into one tile + one DMA (used at
            # the tail, where per-m DMA triggers would serialize).
            js = slice(j * nq + c0, j * nq + c1)
            acc = ps.tile([P, c1 - c0], F32, tag="s2", bufs=2, name="pacc")
            for t in range(CL // P):
                nc.tensor.matmul(
                    acc[:], wpr_sb[:, t:t + 1, m * P:(m + 1) * P],
                    attnT[t][:, js], start=(t == 0), stop=(t == CL // P - 1))
            if gather is not None:
                g_t, m0, mn = gather
                dst = g_t[:, m - m0:m - m0 + 1, :]
            else:
                dst = osp.tile([P, c1 - c0], F16, tag="ost", name="ost")[:]
            if drain == "act":
                nc.scalar.copy(dst, acc[:])
            else:
                nc.vector.tensor_copy(dst, acc[:])
            if gather is None:
                nc.sync.dma_start(outT_d[:, m:m + 1, js], dst)
            elif m == m0 + mn - 1:
                nc.sync.dma_start(outT_d[:, m0:m0 + mn, js], g_t[:])

        nchunks = (nb + BLK - 1) // BLK
        mpc = (C // P) // nchunks      # proj m-steps per chunk
        prev = None    # (unit, oas, pts)
        prev2 = None
        for idx in range(len(units) + 1):
            cur = units[idx] if idx < len(units) else None
            # norm of prev2 first: its reads gate the slot reuse of this
            # iteration's AV writes, so it must precede them in scheduler
            # priority (emitting it later deadlocks the in-order PE queue)
            proj_j = None
            if prev2 is not None:
                emit_norm(prev2[0], prev2[1])
                j2, hp2 = prev2[0]
                if hp2 == HL // HPB - 1:
                    proj_j = j2
            oas_prev = None
            if prev is not None:
                oas_prev = [ps.tile([P, nq], F32, tag="oas", bufs=4,
                                    name=f"oa{s_}") for s_ in range(HPB)]
            pts = {}
            for ib in range(nchunks):
                blk = range(ib * BLK, min((ib + 1) * BLK, nb))
                if cur is not None:
                    for i in blk:
                        pts[i] = emit_s(cur, i)
                if prev is not None:
                    for i in blk:
                        emit_av(prev[0], oas_prev, prev[2], i)
                # proj of the finished block rides along, a couple of
                # m-slices per chunk, so its PSUM slot rotation (and the
                # drain DVE work) spreads across the unit instead of
                # stalling the PE in one burst
                if proj_j is not None:
                    for m in range(ib * mpc, (ib + 1) * mpc):
                        emit_proj_m(proj_j, m)
            if proj_j is not None:
                for m in range(nchunks * mpc, C // P):
                    emit_proj_m(proj_j, m)
            prev2 = (prev[0], oas_prev) if prev is not None else None
            prev = (cur, None, pts) if cur is not None else None

        # tail: the last unit's norm has nothing left to hide under, so
        # run it in column halves with the final projection interleaved;
        # the drains go to the (now idle) scalar engine so they don't
        # serialize behind the norm chain on DVE, and are gathered into
        # 4-m-slice tiles so only two DMA triggers trail each half
        MG = 4
        for c0, c1 in ((0, nq // 2), (nq // 2, nq)):
            emit_norm(prev2[0], prev2[1], c0, c1)
            for m0 in range(0, C // P, MG):
                g_t = osp.tile([P, MG, c1 - c0], F16, tag="ostg", bufs=2,
                               name="ostg")
                for m in range(m0, m0 + MG):
                    emit_proj_m(nj - 1, m, c0, c1, drain="act",
                                gather=(g_t, m0, MG))

    nc.compile()
    return nc


_NC_CACHE = {}


def _get_nc(n=N, nq=NQ):
    key = (n, nq)
    if key not in _NC_CACHE:
        _NC_CACHE[key] = build(n, nq)
    return _NC_CACHE[key]


def make_in_maps(x, w_qkv, b_qkv, q_w, k_w, w_proj, b_proj):
    """Shard full inputs into per-core in_maps (host side)."""
    def ktile(a):
        # [C, W] -> [P, C//P, W] so one DMA fills the whole SBUF tile
        return np.ascontiguousarray(
            a.reshape(a.shape[0] // P, P, a.shape[1]).transpose(1, 0, 2))

    in_maps = []
    for cid in range(NCORES):
        b, g = cid // GROUPS, cid % GROUPS
        c0 = g * CL
        xT = np.ascontiguousarray(x[b].T)
        w_qk = np.concatenate([w_qkv[:, c0:c0 + CL],
                               w_qkv[:, C + c0:C + c0 + CL]], axis=1)
        w_v = w_qkv[:, 2 * C + c0:2 * C + c0 + CL]
        w_pr = w_proj[c0:c0 + CL, :]
        b_qk = np.stack([b_qkv[c0 + m * P:c0 + (m + 1) * P] for m in range(2)]
                        + [b_qkv[C + c0 + m * P:C + c0 + (m + 1) * P]
                           for m in range(2)], axis=1)
        qkw = np.stack([np.tile(q_w, HPB), np.tile(q_w, HPB),
                        np.tile(k_w, HPB), np.tile(k_w, HPB)], axis=1)
        in_maps.append({
            "xT": ktile(xT).astype(np.float16),
            "w_qk": ktile(w_qk).astype(np.float16),
            "w_v": ktile(w_v).astype(np.float16),
            "w_pr": ktile(w_pr).astype(np.float16),
            "b_qk": np.ascontiguousarray(b_qk).astype(np.float32),
            "qkw": np.ascontiguousarray(qkw).astype(np.float32),
        })
    return in_maps


def kernel(x, w_qkv, b_qkv, q_w, k_w, w_proj, b_proj, _trace=False):
    x = np.asarray(x)
    n = x.shape[1]
    nc = _get_nc(n, NQ if n % NQ == 0 else P)
    in_maps = make_in_maps(np.asarray(x, np.float32), np.asarray(w_qkv, np.float32),
                           np.asarray(b_qkv, np.float32), np.asarray(q_w, np.float32),
                           np.asarray(k_w, np.float32), np.asarray(w_proj, np.float32),
                           np.asarray(b_proj, np.float32))
    res = run_bass_kernel_spmd(nc, in_maps, core_ids=list(range(NCORES)),
                               trace=_trace)
    # TP unshard: sum the 4 head-group fp16 partials per batch, transpose,
    # and add the host-folded bias (b_proj plus b_v @ w_proj: attention
    # rows sum to 1, so the v bias is additive on attn_out)
    bias = (np.asarray(b_proj, np.float32)
            + np.asarray(b_qkv[2 * C:3 * C], np.float32)
            @ np.asarray(w_proj, np.float32))
    def unshard(r):
        # outT is [P, C//P, n]: channel c = m*P + p
        a = r.astype(np.float32)
        return a.swapaxes(0, 1).reshape(C, -1)

    out = np.stack([
        sum(unshard(res.results[b * GROUPS + g]["outT"])
            for g in range(GROUPS)).T + bias
        for b in range(B)
    ]).astype(np.float32)
    if _trace:
        return out, res
    return out
